# revision 19
# baseline (speedup 1.0000x reference)
"""Trainium2 Bass kernel for BiasedMHA (B=4, N=1024, C=1024, H=16, D=64).

Sharding: 8 cores = 4 batches x 2 head-halves (8 heads each).
Each core computes q/k/v projections for its head slice, biased softmax
attention, and its partial o_proj.  Host sums the two partials per batch
and adds the (bo + bv @ Wo.T) constant.

v2: all matmul operands fp16 (PE streams 2-byte moving operands at
1 cycle/column vs 2 for fp32/fp32r; accumulation stays fp32 in PSUM).
attn_bias is added by an identity-matmul accumulating into the score
PSUM (start=True), so the Vector engine is out of the attention inner
loop entirely; exp reads PSUM directly.

Layouts (host-prepped, contraction-on-partitions):
  xT    [C, N]  fp16 : x[b].T
  wqT   [C, 512] fp16: Wq[rows,:].T * scale (softmax scale folded here)
  wkT/wvT [C, 512] fp16
  woT   [512, C] fp16: Wo[:, cols].T
  biasT [8, N, N] fp16: attn_bias[b, heads].transpose(0,2,1)  ([h, m, n])
  bqr/bkr [1, 512] fp16: bias rows, added via K=1 matmuls with a ones row
  ident [128, 128] fp16: identity (stationary operand of the bias-add mm)
  madd  [128, 8] fp32: additive mask (-1e30 where attn_mask==0), m-tiled

Attention per head: S^T[m, n] accumulates ident.T @ biasT (start=True)
then k^T(d,m).T @ q^T(d,n) (head pairs row-packed, K=64 at array rows
0-63/64-127).  Exp on ACT straight from PSUM (mask as per-partition
bias; no max-subtraction -- scores are O(+-7)).  P@V consumes expS^T
directly; v carries a ones column per head so PV row 64 is the softmax
denominator.  Normalize via reciprocal_approx_fast + ones-broadcast
matmul, multiply into attT[j, n] (fp16), then o_proj; partials returned
fp16 and summed on host in fp32.
"""

import sys

if "/opt/trn_rl_repo" not in sys.path:
    sys.path.insert(0, "/opt/trn_rl_repo")

from contextlib import ExitStack

import numpy as np

B, N, C, H = 4, 1024, 1024, 16
D = C // H            # 64
HL = H // 2           # 8 local heads per core
JL = HL * D           # 512 local head dims
NT = N // 128         # 8 seq tiles
CT = C // 128         # 8 contraction tiles
SCALE = D ** (-0.5)

_prog_cache = {}


def build_program():
    import concourse.tile as tile
    from concourse import bacc, mybir
    f32 = mybir.dt.float32
    f16 = mybir.dt.float16

    nc = bacc.Bacc("TRN2", target_bir_lowering=False, debug=False,
                   enable_asserts=False, num_devices=8)

    xT = nc.dram_tensor("xT", [C, N], f16, kind="ExternalInput").ap()
    wqT = nc.dram_tensor("wqT", [C, JL], f16, kind="ExternalInput").ap()
    wkT = nc.dram_tensor("wkT", [C, JL], f16, kind="ExternalInput").ap()
    wvT = nc.dram_tensor("wvT", [C, JL], f16, kind="ExternalInput").ap()
    woT = nc.dram_tensor("woT", [JL, C], f16, kind="ExternalInput").ap()
    bqr = nc.dram_tensor("bqr", [1, JL], f16, kind="ExternalInput").ap()
    bkr = nc.dram_tensor("bkr", [1, JL], f16, kind="ExternalInput").ap()
    ident = nc.dram_tensor("ident", [128, 128], f16, kind="ExternalInput").ap()
    biasT = nc.dram_tensor("biasT", [HL, N, N], f16, kind="ExternalInput").ap()
    madd = nc.dram_tensor("madd", [128, NT], f32, kind="ExternalInput").ap()
    outp = nc.dram_tensor("outp", [N, C], f16, kind="ExternalOutput").ap()

    Exp = mybir.ActivationFunctionType.Exp
    mult_op = mybir.AluOpType.mult

    with tile.TileContext(nc) as tc, ExitStack() as ctx:
        # ---- pools ----
        resident = ctx.enter_context(tc.tile_pool(name="resident", bufs=1))
        wpool = ctx.enter_context(tc.tile_pool(name="wstream", bufs=10))
        biaspool = ctx.enter_context(tc.tile_pool(name="bias", bufs=10))
        exppool = ctx.enter_context(tc.tile_pool(name="exps", bufs=8))
        outpool = ctx.enter_context(tc.tile_pool(name="outs", bufs=3))
        smallpool = ctx.enter_context(tc.tile_pool(name="small", bufs=3))
        ps_main = ctx.enter_context(
            tc.tile_pool(name="ps_main", bufs=2, space="PSUM"))
        ps_pv = ctx.enter_context(
            tc.tile_pool(name="ps_pv", bufs=2, space="PSUM"))

        # ---- resident tiles ----
        xts = resident.tile([128, CT, N], f16)          # xT tiled on c
        xview = xT.rearrange("(ct p) n -> p ct n", p=128)
        for ct in range(CT):
            # split across DMA queues: per-queue BW is only ~18 GB/s
            for q in range(4):
                sl = slice(q * 256, (q + 1) * 256)
                nc.sync.dma_start(xts[:, ct, sl], xview[:, ct, sl])

        wv_sb = resident.tile([128, CT, JL], f16)       # wvT tiled on c
        wvview = wvT.rearrange("(ct p) j -> p ct j", p=128)
        for ct in range(CT):
            nc.sync.dma_start(wv_sb[:, ct, :], wvview[:, ct, :])

        wo_sb = resident.tile([128, 4, C], f16)         # woT tiled on j
        woview = woT.rearrange("(kt p) c -> p kt c", p=128)
        for kt in range(4):
            nc.sync.dma_start(wo_sb[:, kt, :], woview[:, kt, :])

        id_sb = resident.tile([128, 128], f16)
        nc.sync.dma_start(id_sb[:], ident)

        madd_sb = resident.tile([128, NT], f32)
        nc.sync.dma_start(madd_sb[:], madd)

        bq_sb = resident.tile([1, JL], f16)
        nc.sync.dma_start(bq_sb[:], bqr)
        bk_sb = resident.tile([1, JL], f16)
        nc.sync.dma_start(bk_sb[:], bkr)

        ones_f32 = resident.tile([128, 1], f32)
        nc.vector.memset(ones_f32[:], 1.0)
        ones_row = resident.tile([1, N], f16)
        nc.vector.tensor_copy(
            ones_row[:], ones_f32[0:1, 0:1].to_broadcast([1, N]))

        qT_sb = resident.tile([128, 4, N], f16)         # [j-tile, n]
        kT_sb = resident.tile([128, 4, N], f16)
        v_sb = resident.tile([128, NT, HL * (D + 1)], f16)  # [m-tile, h*65]
        attT_sb = resident.tile([128, 4, N], f16)       # [j-tile, n]

        # ones columns of v (softmax denominator trick)
        for mt in range(NT):
            v4 = v_sb[:, mt, :].rearrange("p (h c) -> p h c", c=D + 1)
            nc.vector.tensor_copy(
                v4[:, :, D:D + 1],
                ones_f32[:, 0:1, None].to_broadcast([128, HL, 1]))

        # ---- phase 1: projections ----
        # q/k transposed: out[j-tile, n] = sum_c wT[c, j] * xT[c, n] (+ bias)
        for (wdram, brow, dest) in ((wqT, bq_sb, qT_sb), (wkT, bk_sb, kT_sb)):
            wview = wdram.rearrange("(ct p) j -> p ct j", p=128)
            for jt in range(4):
                ps = ps_main.tile([128, N], f32, tag="mm")
                for ct in range(CT):
                    w = wpool.tile([128, 128], f16, tag="w")
                    nc.sync.dma_start(w[:], wview[:, ct, jt * 128:(jt + 1) * 128])
                    for nh in range(2):
                        nc.tensor.matmul(
                            ps[:, nh * 512:(nh + 1) * 512],
                            w[:],
                            xts[:, ct, nh * 512:(nh + 1) * 512],
                            start=(ct == 0), stop=False)
                # bias via K=1 matmul: ones over n, bias row over j
                for nh in range(2):
                    nc.tensor.matmul(
                        ps[:, nh * 512:(nh + 1) * 512],
                        brow[0:1, jt * 128:(jt + 1) * 128],
                        ones_row[0:1, nh * 512:(nh + 1) * 512],
                        start=False, stop=True)
                nc.vector.tensor_copy(dest[:, jt, :], ps[:])

        # v normal layout: out[m-tile, j] = sum_c xT[c, m] * wvT[c, j]
        for mt in range(NT):
            ps = ps_main.tile([128, N], f32, tag="mm")
            psv = ps[:, 0:JL]
            for ct in range(CT):
                nc.tensor.matmul(
                    psv,
                    xts[:, ct, mt * 128:(mt + 1) * 128],
                    wv_sb[:, ct, :],
                    start=(ct == 0), stop=(ct == CT - 1))
            v4 = v_sb[:, mt, :].rearrange("p (h c) -> p h c", c=D + 1)
            nc.vector.tensor_copy(
                v4[:, :, 0:D],
                psv.rearrange("p (h c) -> p h c", c=D))

        # ---- phase 2: attention, one head pair at a time ----
        for hp in range(4):
            hA, hB = 2 * hp, 2 * hp + 1
            pv = [ps_pv.tile([128, N], f32, tag="pv", name=f"pv_{hp}_{i}")
                  for i in range(2)]
            for mt in range(NT):
                s_ps = [None, None]
                bt = [None, None]
                for hi, h in enumerate((hA, hB)):
                    b_ = biaspool.tile([128, N], f16, tag="bias",
                                       name=f"bias_{hp}_{mt}_{hi}")
                    nc.sync.dma_start(
                        b_[:], biasT[h, mt * 128:(mt + 1) * 128, :])
                    bt[hi] = b_
                for hi, h in enumerate((hA, hB)):
                    base = hi * 64
                    sp = ps_main.tile([128, N], f32, tag="mm",
                                      name=f"s_{hp}_{mt}_{hi}")
                    s_ps[hi] = sp
                    for nh in range(2):
                        sl = slice(nh * 512, (nh + 1) * 512)
                        # bias first (start=True), then S accumulates
                        nc.tensor.matmul(sp[:, sl], id_sb[:], bt[hi][:, sl],
                                         start=True, stop=False)
                        nc.tensor.matmul(
                            sp[:, sl],
                            kT_sb[base:base + 64, hp,
                                  mt * 128:(mt + 1) * 128],
                            qT_sb[base:base + 64, hp, sl],
                            start=False, stop=True)
                for hi, h in enumerate((hA, hB)):
                    et = exppool.tile([128, N], f16, tag="exp")
                    nc.scalar.activation(et[:], s_ps[hi][:], Exp,
                                         bias=madd_sb[:, mt:mt + 1])
                    vx = v_sb[:, mt, h * 65:(h + 1) * 65]
                    for nh in range(2):
                        nc.tensor.matmul(
                            pv[hi][0:65, nh * 512:(nh + 1) * 512],
                            vx,
                            et[:, nh * 512:(nh + 1) * 512],
                            start=(mt == 0), stop=(mt == NT - 1))
            # normalize: attT[j, n] = pv[0:64] * (1 / pv[64])
            for hi, h in enumerate((hA, hB)):
                # custom-DVE ops read garbage from PSUM on HW; bounce via SBUF
                den = smallpool.tile([1, N], f32, tag="den")
                nc.vector.tensor_copy(den[:], pv[hi][64:65, :])
                recip32 = smallpool.tile([1, N], f32, tag="recip32")
                nc.vector.reciprocal_approx_fast(out=recip32[:], in_=den[:])
                recip16 = smallpool.tile([1, N], f16, tag="recip16")
                nc.vector.tensor_copy(recip16[:], recip32[:])
                # broadcast recip into the unused partitions 64..128 of the
                # pv tile itself (frees ps_main for next-pair lookahead)
                for nh in range(2):
                    nc.tensor.matmul(
                        pv[hi][64:128, nh * 512:(nh + 1) * 512],
                        ones_row[0:1, 0:64],
                        recip16[0:1, nh * 512:(nh + 1) * 512],
                        start=True, stop=True, tile_position=(0, 64))
                bc_sb = smallpool.tile([64, N], f16, tag="bcast")
                nc.vector.tensor_copy(bc_sb[:], pv[hi][64:128, :])
                nc.vector.tensor_tensor(
                    attT_sb[hi * 64:hi * 64 + 64, hp, :],
                    pv[hi][0:64, :], bc_sb[:], mult_op)

        # ---- phase 3: o_proj partial ----
        for nt in range(NT):
            ps = ps_main.tile([128, N], f32, tag="mm")
            for ch in range(2):
                for kt in range(4):
                    nc.tensor.matmul(
                        ps[:, ch * 512:(ch + 1) * 512],
                        attT_sb[:, kt, nt * 128:(nt + 1) * 128],
                        wo_sb[:, kt, ch * 512:(ch + 1) * 512],
                        start=(kt == 0), stop=(kt == 3))
            ot = outpool.tile([128, N], f16, tag="out")
            nc.vector.tensor_copy(ot[:], ps[:])
            for q in range(2):
                sl = slice(q * 512, (q + 1) * 512)
                nc.sync.dma_start(outp[nt * 128:(nt + 1) * 128, sl], ot[:, sl])

    nc.compile()
    return nc


def get_program():
    if "nc" not in _prog_cache:
        _prog_cache["nc"] = build_program()
    return _prog_cache["nc"]


def make_in_maps(x, attn_bias, attn_mask, Wq, bq, Wk, bk, Wv, bv, Wo, bo):
    """Host-side shard + layout prep.  Returns (in_maps, const) where
    const[c_out] = bo + bv @ Wo.T must be added to the gathered output."""
    x = np.asarray(x, np.float32)
    attn_bias = np.asarray(attn_bias, np.float32)
    attn_mask = np.asarray(attn_mask)
    Wq = np.asarray(Wq, np.float32)
    Wk = np.asarray(Wk, np.float32)
    Wv = np.asarray(Wv, np.float32)
    Wo = np.asarray(Wo, np.float32)
    bq = np.asarray(bq, np.float32)
    bk = np.asarray(bk, np.float32)
    bv = np.asarray(bv, np.float32)
    bo = np.asarray(bo, np.float32)

    const = bo + bv @ Wo.T
    ident = np.eye(128, dtype=np.float16)

    xTs = [np.ascontiguousarray(x[b].T).astype(np.float16) for b in range(B)]
    madds = []
    for b in range(B):
        ma = np.where(attn_mask[b] == 0, np.float32(-1e30), np.float32(0.0))
        madds.append(np.ascontiguousarray(ma.reshape(NT, 128).T))

    in_maps = []
    for core in range(8):
        b, half = divmod(core, 2)
        rows = slice(half * JL, (half + 1) * JL)
        wqT = np.ascontiguousarray(
            (Wq[rows, :] * np.float32(SCALE)).T).astype(np.float16)
        wkT = np.ascontiguousarray(Wk[rows, :].T).astype(np.float16)
        wvT = np.ascontiguousarray(Wv[rows, :].T).astype(np.float16)
        woT = np.ascontiguousarray(Wo[:, rows].T).astype(np.float16)
        bqr = (bq[rows] * np.float32(SCALE)).reshape(1, JL).astype(np.float16)
        bkr = bk[rows].reshape(1, JL).astype(np.float16)
        bT = np.ascontiguousarray(
            attn_bias[b, half * HL:(half + 1) * HL].transpose(0, 2, 1)
        ).astype(np.float16)
        in_maps.append({
            "xT": xTs[b], "wqT": wqT, "wkT": wkT, "wvT": wvT, "woT": woT,
            "bqr": bqr, "bkr": bkr, "ident": ident, "biasT": bT,
            "madd": madds[b],
        })
    return in_maps, const


def gather(results, const):
    out = np.empty((B, N, C), np.float32)
    for b in range(B):
        out[b] = results[2 * b]["outp"].astype(np.float32) \
            + results[2 * b + 1]["outp"].astype(np.float32) \
            + const[None, :]
    return out


def kernel(**inputs):
    from concourse.bass_utils import run_bass_kernel_spmd
    nc = get_program()
    in_maps, const = make_in_maps(**inputs)
    res = run_bass_kernel_spmd(nc, in_maps, core_ids=list(range(8)))
    return gather(res.results, const)


# revision 20
# speedup vs baseline: 1.0820x; 1.0820x over previous
"""Trainium2 Bass kernel for BiasedMHA (B=4, N=1024, C=1024, H=16, D=64).

Sharding: 8 cores = 4 batches x 2 head-halves (8 heads each).
Each core computes q/k/v projections for its head slice, biased softmax
attention, and its partial o_proj.  Host sums the two partials per batch
and adds the (bo + bv @ Wo.T) constant.

v2: all matmul operands fp16 (PE streams 2-byte moving operands at
1 cycle/column vs 2 for fp32/fp32r; accumulation stays fp32 in PSUM).
attn_bias is added by an identity-matmul accumulating into the score
PSUM (start=True), so the Vector engine is out of the attention inner
loop entirely; exp reads PSUM directly.

Layouts (host-prepped, contraction-on-partitions):
  xT    [C, N]  fp16 : x[b].T
  wqT   [C, 512] fp16: Wq[rows,:].T * scale (softmax scale folded here)
  wkT/wvT [C, 512] fp16
  woT   [512, C] fp16: Wo[:, cols].T
  biasT [8, N, N] fp16: attn_bias[b, heads].transpose(0,2,1)  ([h, m, n])
  bqr/bkr [1, 512] fp16: bias rows, added via K=1 matmuls with a ones row
  ident [128, 128] fp16: identity (stationary operand of the bias-add mm)
  madd  [128, 8] fp32: additive mask (-1e30 where attn_mask==0), m-tiled

Attention per head: S^T[m, n] accumulates ident.T @ biasT (start=True)
then k^T(d,m).T @ q^T(d,n) (head pairs row-packed, K=64 at array rows
0-63/64-127).  Exp on ACT straight from PSUM (mask as per-partition
bias; no max-subtraction -- scores are O(+-7)).  P@V consumes expS^T
directly; v carries a ones column per head so PV row 64 is the softmax
denominator.  Normalize via reciprocal_approx_fast + ones-broadcast
matmul, multiply into attT[j, n] (fp16), then o_proj; partials returned
fp16 and summed on host in fp32.
"""

import sys

if "/opt/trn_rl_repo" not in sys.path:
    sys.path.insert(0, "/opt/trn_rl_repo")

from contextlib import ExitStack

import numpy as np

B, N, C, H = 4, 1024, 1024, 16
D = C // H            # 64
HL = H // 2           # 8 local heads per core
JL = HL * D           # 512 local head dims
NT = N // 128         # 8 seq tiles
CT = C // 128         # 8 contraction tiles
SCALE = D ** (-0.5)

_prog_cache = {}


def build_program():
    import concourse.tile as tile
    from concourse import bacc, mybir
    f32 = mybir.dt.float32
    f16 = mybir.dt.float16

    nc = bacc.Bacc("TRN2", target_bir_lowering=False, debug=False,
                   enable_asserts=False, num_devices=8)

    xT = nc.dram_tensor("xT", [C, N], f16, kind="ExternalInput").ap()
    wqT = nc.dram_tensor("wqT", [C, JL], f16, kind="ExternalInput").ap()
    wkT = nc.dram_tensor("wkT", [C, JL], f16, kind="ExternalInput").ap()
    wvT = nc.dram_tensor("wvT", [C, JL], f16, kind="ExternalInput").ap()
    woT = nc.dram_tensor("woT", [JL, C], f16, kind="ExternalInput").ap()
    bqr = nc.dram_tensor("bqr", [1, JL], f16, kind="ExternalInput").ap()
    bkr = nc.dram_tensor("bkr", [1, JL], f16, kind="ExternalInput").ap()
    ident = nc.dram_tensor("ident", [128, 128], f16, kind="ExternalInput").ap()
    biasT = nc.dram_tensor("biasT", [HL, N, N], f16, kind="ExternalInput").ap()
    madd = nc.dram_tensor("madd", [128, NT], f32, kind="ExternalInput").ap()
    outp = nc.dram_tensor("outp", [N, C], f16, kind="ExternalOutput").ap()

    Exp = mybir.ActivationFunctionType.Exp
    mult_op = mybir.AluOpType.mult

    with tile.TileContext(nc) as tc, ExitStack() as ctx:
        # ---- pools ----
        resident = ctx.enter_context(tc.tile_pool(name="resident", bufs=1))
        biaspool = ctx.enter_context(tc.tile_pool(name="bias", bufs=10))
        exppool = ctx.enter_context(tc.tile_pool(name="exps", bufs=8))
        outpool = ctx.enter_context(tc.tile_pool(name="outs", bufs=3))
        smallpool = ctx.enter_context(tc.tile_pool(name="small", bufs=3))
        ps_main = ctx.enter_context(
            tc.tile_pool(name="ps_main", bufs=2, space="PSUM"))
        ps_pv = ctx.enter_context(
            tc.tile_pool(name="ps_pv", bufs=2, space="PSUM"))

        # ---- resident tiles ----
        xts = resident.tile([128, CT, N], f16)          # xT tiled on c
        nc.sync.dma_start(xts[:], xT.rearrange("(ct p) n -> p ct n", p=128))

        wq_sb = resident.tile([128, CT, JL], f16)       # wqT tiled on c
        nc.sync.dma_start(wq_sb[:], wqT.rearrange("(ct p) j -> p ct j", p=128))
        wk_sb = resident.tile([128, CT, JL], f16)       # wkT tiled on c
        nc.sync.dma_start(wk_sb[:], wkT.rearrange("(ct p) j -> p ct j", p=128))

        wv_sb = resident.tile([128, CT, JL], f16)       # wvT tiled on c
        nc.sync.dma_start(wv_sb[:], wvT.rearrange("(ct p) j -> p ct j", p=128))

        wo_sb = resident.tile([128, 4, C], f16)         # woT tiled on j
        nc.sync.dma_start(wo_sb[:], woT.rearrange("(kt p) c -> p kt c", p=128))

        id_sb = resident.tile([128, 128], f16)
        nc.sync.dma_start(id_sb[:], ident)

        madd_sb = resident.tile([128, NT], f32)
        nc.sync.dma_start(madd_sb[:], madd)

        bq_sb = resident.tile([1, JL], f16)
        nc.sync.dma_start(bq_sb[:], bqr)
        bk_sb = resident.tile([1, JL], f16)
        nc.sync.dma_start(bk_sb[:], bkr)

        ones_f32 = resident.tile([128, 1], f32)
        nc.vector.memset(ones_f32[:], 1.0)
        ones_row = resident.tile([1, N], f16)
        nc.vector.tensor_copy(
            ones_row[:], ones_f32[0:1, 0:1].to_broadcast([1, N]))

        qT_sb = resident.tile([128, 4, N], f16)         # [j-tile, n]
        kT_sb = resident.tile([128, 4, N], f16)
        v_sb = resident.tile([128, NT, HL * (D + 1)], f16)  # [m-tile, h*65]
        attT_sb = resident.tile([128, 4, N], f16)       # [j-tile, n]

        # ones columns of v (softmax denominator trick)
        for mt in range(NT):
            v4 = v_sb[:, mt, :].rearrange("p (h c) -> p h c", c=D + 1)
            nc.vector.tensor_copy(
                v4[:, :, D:D + 1],
                ones_f32[:, 0:1, None].to_broadcast([128, HL, 1]))

        # ---- phase 1: projections ----
        # q/k transposed: out[j-tile, n] = sum_c wT[c, j] * xT[c, n] (+ bias)
        for (wsb, brow, dest) in ((wq_sb, bq_sb, qT_sb), (wk_sb, bk_sb, kT_sb)):
            for jt in range(4):
                ps = ps_main.tile([128, N], f32, tag="mm")
                for ct in range(CT):
                    w = wsb[:, ct, jt * 128:(jt + 1) * 128]
                    for nh in range(2):
                        nc.tensor.matmul(
                            ps[:, nh * 512:(nh + 1) * 512],
                            w[:],
                            xts[:, ct, nh * 512:(nh + 1) * 512],
                            start=(ct == 0), stop=False)
                # bias via K=1 matmul: ones over n, bias row over j
                for nh in range(2):
                    nc.tensor.matmul(
                        ps[:, nh * 512:(nh + 1) * 512],
                        brow[0:1, jt * 128:(jt + 1) * 128],
                        ones_row[0:1, nh * 512:(nh + 1) * 512],
                        start=False, stop=True)
                nc.vector.tensor_copy(dest[:, jt, :], ps[:])

        # v normal layout: out[m-tile, j] = sum_c xT[c, m] * wvT[c, j]
        for mt in range(NT):
            ps = ps_main.tile([128, N], f32, tag="mm")
            psv = ps[:, 0:JL]
            for ct in range(CT):
                nc.tensor.matmul(
                    psv,
                    xts[:, ct, mt * 128:(mt + 1) * 128],
                    wv_sb[:, ct, :],
                    start=(ct == 0), stop=(ct == CT - 1))
            v4 = v_sb[:, mt, :].rearrange("p (h c) -> p h c", c=D + 1)
            nc.vector.tensor_copy(
                v4[:, :, 0:D],
                psv.rearrange("p (h c) -> p h c", c=D))

        # ---- phase 2: attention, one head pair at a time ----
        for hp in range(4):
            hA, hB = 2 * hp, 2 * hp + 1
            pv = [ps_pv.tile([128, N], f32, tag="pv", name=f"pv_{hp}_{i}")
                  for i in range(2)]
            for mt in range(NT):
                s_ps = [None, None]
                bt = [None, None]
                for hi, h in enumerate((hA, hB)):
                    b_ = biaspool.tile([128, N], f16, tag="bias",
                                       name=f"bias_{hp}_{mt}_{hi}")
                    nc.gpsimd.dma_start(
                        b_[:], biasT[h, mt * 128:(mt + 1) * 128, :])
                    bt[hi] = b_
                for hi, h in enumerate((hA, hB)):
                    base = hi * 64
                    sp = ps_main.tile([128, N], f32, tag="mm",
                                      name=f"s_{hp}_{mt}_{hi}")
                    s_ps[hi] = sp
                    for nh in range(2):
                        sl = slice(nh * 512, (nh + 1) * 512)
                        # bias first (start=True), then S accumulates
                        nc.tensor.matmul(sp[:, sl], id_sb[:], bt[hi][:, sl],
                                         start=True, stop=False)
                        nc.tensor.matmul(
                            sp[:, sl],
                            kT_sb[base:base + 64, hp,
                                  mt * 128:(mt + 1) * 128],
                            qT_sb[base:base + 64, hp, sl],
                            start=False, stop=True)
                for hi, h in enumerate((hA, hB)):
                    et = exppool.tile([128, N], f16, tag="exp")
                    nc.scalar.activation(et[:], s_ps[hi][:], Exp,
                                         bias=madd_sb[:, mt:mt + 1])
                    vx = v_sb[:, mt, h * 65:(h + 1) * 65]
                    for nh in range(2):
                        nc.tensor.matmul(
                            pv[hi][0:65, nh * 512:(nh + 1) * 512],
                            vx,
                            et[:, nh * 512:(nh + 1) * 512],
                            start=(mt == 0), stop=(mt == NT - 1))
            # normalize: attT[j, n] = pv[0:64] * (1 / pv[64])
            for hi, h in enumerate((hA, hB)):
                # custom-DVE ops read garbage from PSUM on HW; bounce via SBUF
                den = smallpool.tile([1, N], f32, tag="den")
                nc.vector.tensor_copy(den[:], pv[hi][64:65, :])
                recip32 = smallpool.tile([1, N], f32, tag="recip32")
                nc.vector.reciprocal_approx_fast(out=recip32[:], in_=den[:])
                recip16 = smallpool.tile([1, N], f16, tag="recip16")
                nc.vector.tensor_copy(recip16[:], recip32[:])
                # broadcast recip into the unused partitions 64..128 of the
                # pv tile itself (frees ps_main for next-pair lookahead)
                for nh in range(2):
                    nc.tensor.matmul(
                        pv[hi][64:128, nh * 512:(nh + 1) * 512],
                        ones_row[0:1, 0:64],
                        recip16[0:1, nh * 512:(nh + 1) * 512],
                        start=True, stop=True, tile_position=(0, 64))
                bc_sb = smallpool.tile([64, N], f16, tag="bcast")
                nc.vector.tensor_copy(bc_sb[:], pv[hi][64:128, :])
                nc.vector.tensor_tensor(
                    attT_sb[hi * 64:hi * 64 + 64, hp, :],
                    pv[hi][0:64, :], bc_sb[:], mult_op)

        # ---- phase 3: o_proj partial ----
        for nt in range(NT):
            ps = ps_main.tile([128, N], f32, tag="mm")
            for ch in range(2):
                for kt in range(4):
                    nc.tensor.matmul(
                        ps[:, ch * 512:(ch + 1) * 512],
                        attT_sb[:, kt, nt * 128:(nt + 1) * 128],
                        wo_sb[:, kt, ch * 512:(ch + 1) * 512],
                        start=(kt == 0), stop=(kt == 3))
            ot = outpool.tile([128, N], f16, tag="out")
            nc.vector.tensor_copy(ot[:], ps[:])
            nc.sync.dma_start(outp[nt * 128:(nt + 1) * 128, :], ot[:])

    nc.compile()
    return nc


def get_program():
    if "nc" not in _prog_cache:
        _prog_cache["nc"] = build_program()
    return _prog_cache["nc"]


def make_in_maps(x, attn_bias, attn_mask, Wq, bq, Wk, bk, Wv, bv, Wo, bo):
    """Host-side shard + layout prep.  Returns (in_maps, const) where
    const[c_out] = bo + bv @ Wo.T must be added to the gathered output."""
    x = np.asarray(x, np.float32)
    attn_bias = np.asarray(attn_bias, np.float32)
    attn_mask = np.asarray(attn_mask)
    Wq = np.asarray(Wq, np.float32)
    Wk = np.asarray(Wk, np.float32)
    Wv = np.asarray(Wv, np.float32)
    Wo = np.asarray(Wo, np.float32)
    bq = np.asarray(bq, np.float32)
    bk = np.asarray(bk, np.float32)
    bv = np.asarray(bv, np.float32)
    bo = np.asarray(bo, np.float32)

    const = bo + bv @ Wo.T
    ident = np.eye(128, dtype=np.float16)

    xTs = [np.ascontiguousarray(x[b].T).astype(np.float16) for b in range(B)]
    madds = []
    for b in range(B):
        ma = np.where(attn_mask[b] == 0, np.float32(-1e30), np.float32(0.0))
        madds.append(np.ascontiguousarray(ma.reshape(NT, 128).T))

    in_maps = []
    for core in range(8):
        b, half = divmod(core, 2)
        rows = slice(half * JL, (half + 1) * JL)
        wqT = np.ascontiguousarray(
            (Wq[rows, :] * np.float32(SCALE)).T).astype(np.float16)
        wkT = np.ascontiguousarray(Wk[rows, :].T).astype(np.float16)
        wvT = np.ascontiguousarray(Wv[rows, :].T).astype(np.float16)
        woT = np.ascontiguousarray(Wo[:, rows].T).astype(np.float16)
        bqr = (bq[rows] * np.float32(SCALE)).reshape(1, JL).astype(np.float16)
        bkr = bk[rows].reshape(1, JL).astype(np.float16)
        bT = np.ascontiguousarray(
            attn_bias[b, half * HL:(half + 1) * HL].transpose(0, 2, 1)
        ).astype(np.float16)
        in_maps.append({
            "xT": xTs[b], "wqT": wqT, "wkT": wkT, "wvT": wvT, "woT": woT,
            "bqr": bqr, "bkr": bkr, "ident": ident, "biasT": bT,
            "madd": madds[b],
        })
    return in_maps, const


def gather(results, const):
    out = np.empty((B, N, C), np.float32)
    for b in range(B):
        out[b] = results[2 * b]["outp"].astype(np.float32) \
            + results[2 * b + 1]["outp"].astype(np.float32) \
            + const[None, :]
    return out


def kernel(**inputs):
    from concourse.bass_utils import run_bass_kernel_spmd
    nc = get_program()
    in_maps, const = make_in_maps(**inputs)
    res = run_bass_kernel_spmd(nc, in_maps, core_ids=list(range(8)))
    return gather(res.results, const)
